# revision 1
# baseline (speedup 1.0000x reference)
"""ConvLSTM (T=16, B=4, C=32, HID=64, 64x64, 3x3 convs) on 8 Trainium2 cores.

Decomposition: 8 cores = batch(4) x H-halves(2). Each core owns 32 output rows
and recomputes a shrinking halo (rows 0..47-t at step t) so NO inter-core
communication is needed. The bottom-half cores get vertically flipped inputs
(and dy-flipped conv weights) so all 8 cores run the identical SPMD program.

Conv-as-matmul with dense tap packing: the 9 taps x 32ch (x2h) and 9 taps x
64ch (h2h) of the two 3x3 convs are packed into the 128-partition contraction
dim as shifted copies of the padded image, reaching the theoretical floor of
7 matmuls per 128-out-channel half per 512-pixel chunk:
    X1 = x taps (0,0)(0,1)(0,2)(1,0)   [4x32 rows, shifts 0,1,2,66]
    X2 = x taps (1,1)(1,2)(2,0)(2,1)   [shifts 67,68,132,133]
    HA @ dy*66, dy=0,1,2 = h taps (dy,0)(dy,1)   [2x64 rows, shifts 0,1]
    HB2 = h taps (0,2)(2,2)            [shifts 2,134]
    CMB = x tap (2,2) + h tap (1,2)    [96 rows, shifts 134 / 68]
x blocks are DMA'd from DRAM at shifted offsets (free); h blocks are written
by 5 small SBUF DMAs per chunk. All matmuls accumulate into one PSUM bank at
base partition 0 (mixed-base accumulation hangs the HW). MM_DT selects matmul
precision: bf16 (fast) or float32r (~20x more precise, ~1.7x slower PE).

Perf round 1 (vs 479us baseline):
 - h-halo writes go through a 66-pitch padded staging tile (pad cols kept
   zero) so all 5 per-chunk SBUF DMAs move whole contiguous byte ranges:
   1 descriptor/partition instead of 8 (the 128B-descriptor flood was ~65%
   busy on all 16 SDMA engines and serialized step boundaries).
 - gates are packed [i;o] / [g;f] (host-side channel permutation) so i,o
   share one 128-partition Sigmoid call; f's sigmoid and tanh(c) use ACT's
   cross-partition-base writes (the only engine that allows them) to land
   f on parts 0-63 (next to c) and tanh(c) on parts 64-127 (next to o).
   4 ACT calls/chunk instead of 5; DVE ops all stay same-base.
"""
import sys
import os

for _p in ("/opt/trn_rl_repo", "/root/.axon_site"):
    if _p not in sys.path and os.path.isdir(_p):
        sys.path.append(_p)

import numpy as np

T, B, C, H, W, HID = 16, 4, 32, 64, 64, 64
HP, WP = 49, 66          # padded per-core image: 48 data rows + 1 top pad, 64+2 cols
FLAT = HP * WP           # 3234
NR = 8                   # output rows per chunk (N = NR*64 = 512 <= PSUM bank)
XLEN = 3100              # per-block x DMA length (covers max read f=3099)

MM_DT = os.environ.get("KLSTM_MM_DT", "bf16")   # "bf16" | "f32r"

X_TAPS_A = [(0, 0), (0, 1), (0, 2), (1, 0)]
X_TAPS_B = [(1, 1), (1, 2), (2, 0), (2, 1)]

_CACHE = {}


def _build_program():
    import concourse.mybir as mybir
    import concourse.tile as tile
    from concourse import bacc

    f32 = mybir.dt.float32
    dtm = mybir.dt.bfloat16 if MM_DT == "bf16" else mybir.dt.float32r

    nc = bacc.Bacc("TRN2", target_bir_lowering=False, debug=False, num_devices=8)

    xp_d = nc.dram_tensor("xp", [T, C, FLAT], dtm, kind="ExternalInput")
    wx1_d = nc.dram_tensor("wx1", [128, 256], dtm, kind="ExternalInput")
    wx2_d = nc.dram_tensor("wx2", [128, 256], dtm, kind="ExternalInput")
    wa_d = nc.dram_tensor("wa", [128, 768], dtm, kind="ExternalInput")
    wb2_d = nc.dram_tensor("wb2", [128, 256], dtm, kind="ExternalInput")
    wc_d = nc.dram_tensor("wc", [96, 256], dtm, kind="ExternalInput")
    bias_d = nc.dram_tensor("bias", [128, 2], f32, kind="ExternalInput")
    out_d = nc.dram_tensor("out", [T, HID, 32 * 66], f32, kind="ExternalOutput")

    Sigmoid = mybir.ActivationFunctionType.Sigmoid
    Tanh = mybir.ActivationFunctionType.Tanh

    with tile.TileContext(nc) as tc:
        with tc.tile_pool(name="const", bufs=1) as constp, \
             tc.tile_pool(name="xpool", bufs=4 if MM_DT == "bf16" else 2) as xpool, \
             tc.tile_pool(name="hpool", bufs=1) as hpool, \
             tc.tile_pool(name="cpool", bufs=1) as cpool, \
             tc.tile_pool(name="psum", bufs=4, space="PSUM") as psum, \
             tc.tile_pool(name="ifsp", bufs=6) as ifsp, \
             tc.tile_pool(name="smallp", bufs=6) as smallp:

            wx1_s = constp.tile([128, 256], dtm)
            wx2_s = constp.tile([128, 256], dtm)
            wa_s = constp.tile([128, 768], dtm)
            wb2_s = constp.tile([128, 256], dtm)
            wc_s = constp.tile([96, 256], dtm)
            bias_s = constp.tile([128, 2], f32)
            for s_, d_ in [(wx1_s, wx1_d), (wx2_s, wx2_d), (wa_s, wa_d),
                           (wb2_s, wb2_d), (wc_s, wc_d), (bias_s, bias_d)]:
                nc.sync.dma_start(s_[:], d_[:])

            # HAM warmup: ~4us of junk matmuls during the startup ramp (while
            # the first x tiles load) so step 1's matmuls run at 2.4 GHz.
            wrm = psum.tile([128, 512], f32, tag="psA", name="warm")
            for _ in range(20):
                nc.tensor.matmul(wrm[:, 0:512], wa_s[:, 0:128],
                                 wa_s[:, 0:512], start=True, stop=True)

            # ping-pong h tiles (shifted partition blocks, see module docstring)
            hA = [hpool.tile([128, FLAT], dtm, tag=f"hA{i}", name=f"hA{i}")
                  for i in range(2)]
            hB2 = [hpool.tile([128, FLAT], dtm, tag=f"hB2{i}", name=f"hB2{i}")
                   for i in range(2)]
            # cmb: parts 0-31 x tap (2,2) [per-step], parts 32-95 h tap (1,2)
            # 3-deep so step t+1's x-load never waits on step t-1's matmuls
            cmb = [hpool.tile([96, FLAT], dtm, tag=f"cmb{i}", name=f"cmb{i}")
                   for i in range(3)]
            # 66-pitch hout staging ring: row r = [0, hout(r,:), 0]; pad cols
            # 0/65 and the spare row stay zero forever (memset once).
            NSTG = 8
            stg = [hpool.tile([128, (NR + 1) * 66], dtm, tag=f"stg{i}",
                              name=f"stg{i}") for i in range(NSTG)]
            for t_ in hA + hB2 + cmb + stg:
                nc.gpsimd.memset(t_[:] if MM_DT == "bf16" else t_[:].bitcast(f32),
                                 0.0)

            c_s = cpool.tile([64, 47 * 64], f32)

            def load_x(t):
                # x tiles for step t: TA (4 blocks), TB (4 blocks), cmb x-block
                # (issued from the mostly-idle gpsimd queue, ~one step ahead)
                xa = xpool.tile([128, FLAT], dtm, tag="xa", name="xa")
                xb = xpool.tile([128, FLAT], dtm, tag="xb", name="xb")
                for b3, (dy, dx) in enumerate(X_TAPS_A):
                    s = dy * WP + dx
                    nc.gpsimd.dma_start(xa[32 * b3:32 * b3 + 32, 0:XLEN],
                                        xp_d[t - 1, :, s:s + XLEN])
                for b3, (dy, dx) in enumerate(X_TAPS_B):
                    s = dy * WP + dx
                    nc.gpsimd.dma_start(xb[32 * b3:32 * b3 + 32, 0:XLEN],
                                        xp_d[t - 1, :, s:s + XLEN])
                nc.gpsimd.dma_start(cmb[t % 3][64:96, 0:XLEN],
                                    xp_d[t - 1, :, 134:134 + XLEN])
                return xa, xb

            def rv(tile_ap):
                return tile_ap.rearrange("p (y x) -> p y x", x=WP)

            xload = {1: load_x(1)}
            ctx = {}

            def step_ctx(t):
                if t not in ctx:
                    if t + 1 <= T and t + 1 not in xload:
                        xload[t + 1] = load_x(t + 1)
                    xa, xb = xload[t]
                    ctx[t] = dict(
                        xav=rv(xa[:]), xbv=rv(xb[:]),
                        hAv=rv(hA[(t - 1) % 2][:]),
                        hB2v=rv(hB2[(t - 1) % 2][:]),
                        cmbv=rv(cmb[t % 3][:]),
                        hAc=hA[t % 2], hB2c=hB2[t % 2],
                        cmbn=cmb[(t + 1) % 3],
                    )
                return ctx[t]

            def front_job(t, q):
                cx = step_ctx(t)
                R = 48 - t
                y0 = NR * q
                nr = min(NR, R - y0)
                N = nr * 64
                ps = [psum.tile([128, 512], f32, tag="psA", name="psA"),
                      psum.tile([128, 512], f32, tag="psB", name="psB")]
                for h in range(2):
                    pt = ps[h]
                    hs = h * 128
                    nc.tensor.matmul(pt[:, :N], wx1_s[:, hs:hs + 128],
                                     cx["xav"][:, y0:y0 + nr, 0:64],
                                     start=True, stop=False)
                    nc.tensor.matmul(pt[:, :N], wx2_s[:, hs:hs + 128],
                                     cx["xbv"][:, y0:y0 + nr, 0:64],
                                     start=False, stop=False)
                    if t > 1:
                        for dy in range(3):
                            nc.tensor.matmul(
                                pt[:, :N],
                                wa_s[:, (dy * 2 + h) * 128:(dy * 2 + h + 1) * 128],
                                cx["hAv"][:, y0 + dy:y0 + dy + nr, 0:64],
                                start=False, stop=False)
                        nc.tensor.matmul(pt[:, :N], wb2_s[:, hs:hs + 128],
                                         cx["hB2v"][:, y0:y0 + nr, 0:64],
                                         start=False, stop=False)
                        nc.tensor.matmul(
                            pt[:, :N], wc_s[0:64, hs:hs + 128],
                            cx["hB2v"][0:64, y0 + 1:y0 + 1 + nr, 0:64],
                            start=False, stop=False)
                        nc.tensor.matmul(pt[:, :N], wc_s[64:96, hs:hs + 128],
                                         cx["cmbv"][64:96, y0:y0 + nr, 0:64],
                                         start=False, stop=True)
                    else:
                        nc.tensor.matmul(pt[:, :N], wc_s[64:96, hs:hs + 128],
                                         cx["cmbv"][64:96, y0:y0 + nr, 0:64],
                                         start=False, stop=True)
                # ps[0] = [i; o] -> one merged sigmoid; ps[1] = [g; f]
                io = ifsp.tile([128, 512], f32, tag="io", name="io")
                nc.scalar.activation(io[:, :N], ps[0][0:128, :N], Sigmoid,
                                     bias=bias_s[0:128, 0:1])
                gt = smallp.tile([64, 512], f32, tag="gt")
                nc.scalar.activation(gt[:, :N], ps[1][0:64, :N], Tanh,
                                     bias=bias_s[0:64, 1:2])
                c_sl = c_s[:, y0 * 64:y0 * 64 + N]
                if t == 1:
                    nc.vector.tensor_mul(c_sl, io[0:64, :N], gt[:, :N])
                else:
                    fs_ = ifsp.tile([64, 512], f32, tag="fs", name="fs_")
                    nc.scalar.activation(fs_[:, :N], ps[1][64:128, :N],
                                         Sigmoid, bias=bias_s[64:128, 1:2])
                    t1 = smallp.tile([64, 512], f32, tag="t1")
                    nc.vector.tensor_mul(t1[:, :N], io[0:64, :N], gt[:, :N])
                    nc.vector.tensor_mul(c_sl, fs_[:, :N], c_sl)
                    nc.vector.tensor_add(c_sl, c_sl, t1[:, :N])
                return io

            def tail_job(t, q0, ios, sc):
                # pair tail: tanh(c) + staging + halo/out DMAs for chunks
                # q0..q0+len(ios)-1 at up-to-1024 granularity (fewer, larger
                # DMAs and ACT calls; the front stays 512 for fast PSUM
                # release)
                cx = step_ctx(t)
                R = 48 - t
                y0g = NR * q0
                ng = min(len(ios) * NR, R - y0g)
                Nt = ng * 64
                # tanh(c) cross-written to parts 64-127, next to o
                tc_ = smallp.tile([128, 512], f32, tag="tc")
                nc.scalar.activation(tc_[64:128, :Nt],
                                     c_s[:, y0g * 64:y0g * 64 + Nt], Tanh)
                if t < T or y0g < 32:
                    # hout lands in the padded staging ring (cols 1:65 of
                    # each 66-el row); pad cols stay 0 from the init memset.
                    hst = stg[sc % NSTG][64:128]
                    hst3 = hst.rearrange("p (y x) -> p y x", x=66)
                    for j, io_j in enumerate(ios):
                        nrj = min(NR, ng - NR * j)
                        Nj = nrj * 64
                        o3 = io_j[64:128, :Nj].rearrange("p (y x) -> p y x",
                                                         x=64)
                        t3 = tc_[64:128, 512 * j:512 * j + Nj].rearrange(
                            "p (y x) -> p y x", x=64)
                        nc.vector.tensor_mul(hst3[:, NR * j:NR * j + nrj, 1:65],
                                             o3, t3)
                if t < T:
                    # whole-row contiguous halo copies (1 desc/partition):
                    # dest flat offsets chosen so real data lands exactly
                    # where the old per-block shifted writes put it; spill
                    # elements land only in never-read pad columns.
                    hAc, hB2c, cmbn = cx["hAc"], cx["hB2c"], cx["cmbn"]
                    L = ng * 66
                    d0 = (y0g + 1) * 66
                    nc.sync.dma_start(hAc[0:64, d0:d0 + L], hst[:, 0:L])
                    nc.sync.dma_start(hAc[64:128, d0:d0 + L], hst[:, 1:1 + L])
                    nc.sync.dma_start(hB2c[0:64, d0:d0 + L], hst[:, 2:2 + L])
                    if q0 == 0:
                        nc.sync.dma_start(
                            hB2c[64:128, 0:L - 66], hst[:, 68:2 + L])
                    else:
                        d4 = (y0g - 1) * 66
                        nc.sync.dma_start(
                            hB2c[64:128, d4:d4 + L], hst[:, 2:2 + L])
                if y0g < 32:
                    # out rows via casting SWDGE DMA straight from the
                    # bf16 staging rows (padded 66-el layout; host strips)
                    src = (hst[:, 0:ng * 66] if MM_DT == "bf16"
                           else hst[:, 0:ng * 66].bitcast(f32))
                    nc.gpsimd.dma_start(
                        out_d[t - 1, :, y0g * 66:y0g * 66 + ng * 66], src)

            # cross-step wavefront: step t+1's chunk j is emitted alongside
            # step t's chunk j+LAG, so the next step's matmuls depend only on
            # halo writes that are already emitted (their true inputs) and
            # the PE never stalls at a step boundary.
            LAG = 5
            jobs = []
            nch_of = {}
            for t in range(1, T + 1):
                nch_of[t] = (48 - t + NR - 1) // NR
                for q in range(nch_of[t]):
                    jobs.append((LAG * (t - 1) + q, t, q))
            jobs.sort(key=lambda j: (j[0], j[1]))
            sc = 0
            for _, t, q in jobs:
                io = front_job(t, q)
                tail_job(t, q, [io], sc)
                sc += 1
    nc.compile()
    return nc


def _host_prep(x, w_x2h, b_x2h, w_h2h, b_h2h):
    """Build the 8 per-core input maps."""
    import ml_dtypes
    np_dtm = ml_dtypes.bfloat16 if MM_DT == "bf16" else np.float32

    x = np.ascontiguousarray(np.asarray(x, np.float32))
    w_x2h = np.asarray(w_x2h, np.float32)
    b_x2h = np.asarray(b_x2h, np.float32)
    w_h2h = np.asarray(w_h2h, np.float32)
    b_h2h = np.asarray(b_h2h, np.float32)

    # gate-channel permutation: [i, o, g, f] so psum half0=[i;o], half1=[g;f]
    order = np.r_[0:64, 192:256, 128:192, 64:128]

    bias = np.zeros((128, 2), np.float32)
    bsum = (b_x2h + b_h2h)[order]
    bias[:, 0] = bsum[0:128]
    bias[:, 1] = bsum[128:256]

    in_maps = []
    packed_w = {}
    for parity in range(2):
        wx_f = (w_x2h if parity == 0 else w_x2h[:, :, ::-1, :])[order]
        wh_f = (w_h2h if parity == 0 else w_h2h[:, :, ::-1, :])[order]
        wx1 = np.zeros((128, 2, 128), np.float32)
        wx2 = np.zeros((128, 2, 128), np.float32)
        wa = np.zeros((128, 3, 2, 128), np.float32)
        wb2 = np.zeros((128, 2, 128), np.float32)
        wc = np.zeros((96, 2, 128), np.float32)
        for hh in range(2):
            oc = slice(hh * 128, (hh + 1) * 128)
            for b3, (dy, dx) in enumerate(X_TAPS_A):
                wx1[32 * b3:32 * b3 + 32, hh, :] = wx_f[oc, :, dy, dx].T
            for b3, (dy, dx) in enumerate(X_TAPS_B):
                wx2[32 * b3:32 * b3 + 32, hh, :] = wx_f[oc, :, dy, dx].T
            for dy in range(3):
                for b3 in range(2):
                    wa[64 * b3:64 * b3 + 64, dy, hh, :] = wh_f[oc, :, dy, b3].T
            wb2[0:64, hh, :] = wh_f[oc, :, 0, 2].T
            wb2[64:128, hh, :] = wh_f[oc, :, 2, 2].T
            wc[0:64, hh, :] = wh_f[oc, :, 1, 2].T
            wc[64:96, hh, :] = wx_f[oc, :, 2, 2].T
        packed_w[parity] = tuple(
            np.ascontiguousarray(a.reshape(a.shape[0], -1).astype(np_dtm))
            for a in (wx1, wx2, wa, wb2, wc))

    for core in range(8):
        b, parity = core // 2, core % 2
        xv = x[:, b]
        if parity == 1:
            xv = xv[:, :, ::-1, :]
        xp = np.zeros((T, C, HP, WP), np.float32)
        xp[:, :, 1:49, 1:65] = xv[:, :, 0:48, :]
        wx1, wx2, wa, wb2, wc = packed_w[parity]
        in_maps.append({
            "xp": np.ascontiguousarray(xp.reshape(T, C, FLAT).astype(np_dtm)),
            "wx1": wx1, "wx2": wx2, "wa": wa, "wb2": wb2, "wc": wc,
            "bias": bias,
        })
    return in_maps


def kernel(x, w_x2h, b_x2h, w_h2h, b_h2h, _trace=False, _tmpdir=None):
    from concourse.bass_utils import run_bass_kernel_spmd

    if "nc" not in _CACHE:
        _CACHE["nc"] = _build_program()
    nc = _CACHE["nc"]

    in_maps = _host_prep(x, w_x2h, b_x2h, w_h2h, b_h2h)
    kw = {}
    if _trace:
        kw = dict(trace=True, tmpdir=_tmpdir)
    res = run_bass_kernel_spmd(nc, in_maps, core_ids=list(range(8)), **kw)

    full = np.zeros((T, B, HID, H, W), np.float32)
    for core in range(8):
        b, parity = core // 2, core % 2
        out = res.results[core]["out"].reshape(T, HID, 32, 66)[:, :, :, 1:65]
        if parity == 0:
            full[:, b, :, 0:32] = out
        else:
            full[:, b, :, 32:64] = out[:, :, ::-1, :]
    if _trace:
        return full, res
    return full

